# revision 1
# baseline (speedup 1.0000x reference)
"""Residual VQ (Mimi) kernel for 8x TRN2 NeuronCores.

Data-parallel over time: each core processes T/8 = 4096 timesteps.

Per-core algorithm (matches jax fp32 reference bit-closely):
  r_T = (x @ w_in.T).T          kept transposed [256, 4096] as 2x32 [128,128] tiles
  for q in 8 codebooks:
    psum    = 2*r.e_k - etilde_k          (PE: fp32 cross + bf16 aug row)
    s1      = psum - x_sq                 (ACT Identity with per-partition bias)
            = -(x_sq - 2*r.e + etilde) = -dist
    argmin  = max8 + max_index over s1    (DVE; first-index tie-break == jnp.argmin)
    quant   = emb[idx] gather             (indirect DMA)
    r -= quant; out += quant              (PE transpose + DVE, transposed layout)
    x_sq_next = dist_min = -max(s1)       (bias for next layer = max(s1) directly)
  y = out_T.T @ w_out.T

etilde = e_sq rounded to the 2^-17 grid; since fl(x_sq - 2c) lands on that grid
(x_sq in [64,128)), adding etilde is exact and commutes into the PE accumulation,
reproducing the reference's fl(fl(x_sq - 2c) + e_sq) rounding (validated: 1
argmin flip in 262144 vs fp32 reference).
"""
import numpy as np

import concourse.bacc as bacc
import concourse.bass as bass
import concourse.mybir as mybir
import concourse.tile as tile
from concourse.bass_utils import run_bass_kernel_spmd
from concourse.masks import make_identity

F32 = mybir.dt.float32
BF16 = mybir.dt.bfloat16
U32 = mybir.dt.uint32

T, D_IN, D_CB, K, Q = 32768, 512, 256, 2048, 8
import os
NO_GATHER = os.environ.get("VQ_NO_GATHER", "0") == "1"
N_CORES = 8
T_LOC = T // N_CORES          # 4096
NT = T_LOC // 128             # 32 t-tiles
P = 128

Act = mybir.ActivationFunctionType
Alu = mybir.AluOpType


def _build(reps=1):
    nc = bacc.Bacc(None, target_bir_lowering=False, num_swdge_queues=4)

    x = nc.declare_dram_parameter("x", [T_LOC, D_IN], F32, isOutput=False)
    w_in = nc.declare_dram_parameter("w_in", [D_CB, D_IN], F32, isOutput=False)
    w_out = nc.declare_dram_parameter("w_out", [D_IN, D_CB], F32, isOutput=False)
    emb = nc.declare_dram_parameter("emb", [Q * K, D_CB], F32, isOutput=False)
    y = nc.declare_dram_parameter("y", [T_LOC, D_IN], F32, isOutput=True)

    with tile.TileContext(nc) as tc:
      for rep in range(reps):
        R = f"r{rep}_"
        with (
            tc.tile_pool(name=R+"const", bufs=1) as constp,
            tc.tile_pool(name=R+"state", bufs=1) as state,
            tc.tile_pool(name=R+"layer", bufs=2) as layer,
            tc.tile_pool(name=R+"lscratch", bufs=1) as lscratch,
            tc.tile_pool(name=R+"work", bufs=2) as work,
            tc.tile_pool(name=R+"smalls", bufs=4) as smalls,
            tc.tile_pool(name=R+"pdist", bufs=4, space="PSUM") as pdist,
            tc.tile_pool(name=R+"pqt", bufs=2, space="PSUM") as pqt,
            tc.tile_pool(name=R+"paux", bufs=2, space="PSUM") as paux,
        ):
            ident = constp.tile([P, P], F32, tag="ident")
            make_identity(nc, ident[:])
            ones1 = constp.tile([1, P], BF16, tag="ones1")
            nc.gpsimd.memset(ones1[:], 1.0)
            bias_magic = constp.tile([P, 1], F32, tag="bias_magic")
            nc.gpsimd.memset(bias_magic[:], float(2.0 ** 23))
            bias_64 = constp.tile([P, 1], F32, tag="bias_64")
            nc.gpsimd.memset(bias_64[:], 64.0)
            bias_128 = constp.tile([P, 1], F32, tag="bias_128")
            nc.gpsimd.memset(bias_128[:], 128.0)

            w_in_T = constp.tile([P, 4, D_CB], F32, tag="w_in_T")   # [din_p, din_c, dcb]
            w_out_T = constp.tile([P, 2, D_IN], F32, tag="w_out_T")  # [dcb_p, dcb_c, n]

            # r_T, out_T: transposed state, per (dcb-chunk m, t-tile)
            rT = [[state.tile([P, P], F32, tag=f"rT{m}_{t}", name=R+f"rT{m}_{t}")
                   for t in range(NT)] for m in range(2)]
            outT = [[state.tile([P, P], F32, tag=f"oT{m}_{t}", name=R+f"oT{m}_{t}")
                     for t in range(NT)] for m in range(2)]
            # negative x_sq bias, ping-pong across layers
            nxsq = [[state.tile([P, 1], F32, tag=f"nx{s}_{t}", name=R+f"nx{s}_{t}")
                     for t in range(NT)] for s in range(2)]
            augw = [state.tile([2, P], BF16, tag=f"augw_{t}", name=R+f"augw_{t}")
                    for t in range(NT)]
            for t in range(NT):
                nc.gpsimd.memset(augw[t][0:1, :], 1.0)

            # ---------------- init: weight transposes ----------------
            with tc.tile_pool(name=R+"initp", bufs=1) as initp:
                wtmp = initp.tile([P, 2, D_IN], F32, tag="wtmp")
                nc.sync.dma_start(wtmp[:], w_in[:].rearrange("(c p) d -> p c d", p=P))
                for ci in range(4):
                    for m in range(2):
                        tp = paux.tile([P, P], F32, tag="tp")
                        nc.tensor.transpose(tp[:], wtmp[:, m, ci * P:(ci + 1) * P], ident[:])
                        nc.scalar.activation(w_in_T[:, ci, m * P:(m + 1) * P], tp[:], Act.Copy)
                wtmp2 = initp.tile([P, 4, D_CB], F32, tag="wtmp")
                nc.sync.dma_start(wtmp2[:], w_out[:].rearrange("(c p) d -> p c d", p=P))
                for ci in range(4):
                    for m in range(2):
                        tp = paux.tile([P, P], F32, tag="tp")
                        nc.tensor.transpose(tp[:], wtmp2[:, ci, m * P:(m + 1) * P], ident[:])
                        nc.scalar.activation(w_out_T[:, m, ci * P:(ci + 1) * P], tp[:], Act.Copy)

                # ---------------- init: x -> r0_T, x_sq ----------------
                for b in range(8):  # 512-t blocks
                    xblk = initp.tile([P, 4, D_IN], F32, tag="xblk")
                    nc.sync.dma_start(
                        xblk[:], x[b * 512:(b + 1) * 512, :].rearrange("(c p) d -> p c d", p=P))
                    xT = initp.tile([P, 4, 512], F32, tag="xT")  # [din_p, din_c, t_in_blk]
                    for tb in range(4):
                        for db in range(4):
                            tp = paux.tile([P, P], F32, tag="tp")
                            nc.tensor.transpose(tp[:], xblk[:, tb, db * P:(db + 1) * P], ident[:])
                            nc.scalar.activation(xT[:, db, tb * P:(tb + 1) * P], tp[:], Act.Copy)
                    # r0_T chunks
                    for m in range(2):
                        pr = pdist.tile([P, 512], F32, tag="pd")
                        for ci in range(4):
                            nc.tensor.matmul(pr[:], w_in_T[:, ci, m * P:(m + 1) * P],
                                             xT[:, ci, :], start=(ci == 0), stop=(ci == 3))
                        for tb in range(4):
                            nc.scalar.activation(rT[m][b * 4 + tb][:],
                                                 pr[:, tb * P:(tb + 1) * P], Act.Copy)
                    # r0 natural per t-subtile -> x_sq
                    for tb in range(4):
                        t = b * 4 + tb
                        pn = paux.tile([P, D_CB], F32, tag="tp")
                        for ci in range(4):
                            nc.tensor.matmul(pn[:], xT[:, ci, tb * P:(tb + 1) * P],
                                             w_in_T[:, ci, :], start=(ci == 0), stop=(ci == 3))
                        sq = initp.tile([P, D_CB], F32, tag="sq")
                        nc.scalar.activation(sq[:], pn[:], Act.Square)
                        xs = smalls.tile([P, 1], F32, tag="xs")
                        nc.vector.tensor_reduce(xs[:], sq[:], axis=mybir.AxisListType.X,
                                                op=Alu.add)
                        nc.scalar.activation(nxsq[0][t][:], xs[:], Act.Copy, scale=-1.0)
                    for m in range(2):
                        for tb in range(4):
                            nc.vector.memzero(outT[m][b * 4 + tb][:])

            # ---------------- main: 8 codebook layers ----------------
            for q in range(Q):
                # layer prep: e2T (transposed, x2), etilde row (bf16)
                estage = lscratch.tile([P, 16, D_CB], F32, tag="estage",
                                             name=R+f"estage{q}")
                nc.sync.dma_start(
                    estage[:], emb[q * K:(q + 1) * K, :].rearrange("(c p) d -> p c d", p=P))
                e2T = [layer.tile([P, K], F32, tag=f"e2T{m}", name=R+f"e2T{m}_{q}")
                       for m in range(2)]
                for c in range(16):
                    for m in range(2):
                        tp = paux.tile([P, P], F32, tag="tp")
                        nc.tensor.transpose(tp[:], estage[:, c, m * P:(m + 1) * P], ident[:])
                        nc.scalar.activation(e2T[m][:, c * P:(c + 1) * P], tp[:], Act.Copy,
                                             scale=2.0)
                esq = smalls.tile([P, 16], F32, tag="esq")
                for c in range(16):
                    sqc = lscratch.tile([P, D_CB], F32, tag="sqc", name=R+f"sqc{q}_{c}")
                    nc.scalar.activation(sqc[:], estage[:, c, :], Act.Square)
                    nc.vector.tensor_reduce(esq[:, c:c + 1], sqc[:],
                                            axis=mybir.AxisListType.X, op=Alu.add)
                tpe = paux.tile([16, P], F32, tag="tp")
                nc.tensor.transpose(tpe[:], esq[:], ident[:])
                # grid-round e_sq to 2^-17 and 2^-16 (RNE via +2^23 magic), negate.
                # Row 0 of eneg: -etilde17 (applied to every row); row 1:
                # -(etilde16 - etilde17), applied only where x_sq >= 128 (the
                # [128,256) binade rounds dist at 2^-16).
                g17inv, g17 = float(2.0 ** 17), float(2.0 ** -17)
                g16inv, g16 = float(2.0 ** 16), float(2.0 ** -16)
                q1 = smalls.tile([16, P], F32, tag="q1")
                nc.scalar.activation(q1[:], tpe[:], Act.Identity, scale=g17inv,
                                     bias=bias_magic[:16, :])
                q2 = smalls.tile([16, P], F32, tag="q2")
                nc.scalar.activation(q2[:], q1[:], Act.Identity, scale=-g17,
                                     bias=bias_64[:16, :])
                q1b = smalls.tile([16, P], F32, tag="q1b")
                nc.scalar.activation(q1b[:], tpe[:], Act.Identity, scale=g16inv,
                                     bias=bias_magic[:16, :])
                q2c = smalls.tile([16, P], F32, tag="q2c")
                nc.scalar.activation(q2c[:], q1b[:], Act.Identity, scale=-g16,
                                     bias=bias_128[:16, :])
                qv = smalls.tile([16, P], F32, tag="qv")
                nc.vector.tensor_tensor(qv[:], q2c[:], q2[:], op=Alu.subtract)
                q2b = smalls.tile([16, P], BF16, tag="q2b")
                nc.vector.tensor_copy(q2b[:], q2[:])
                qvb = smalls.tile([16, P], BF16, tag="qvb")
                nc.vector.tensor_copy(qvb[:], qv[:])
                eneg = layer.tile([2, K], BF16, tag="eneg")
                nc.sync.dma_start(eneg[0:1, :], q2b[:])
                nc.sync.dma_start(eneg[1:2, :], qvb[:])

                cur, nxt = nxsq[q % 2], nxsq[(q + 1) % 2]
                for t in range(NT):
                    bflag = smalls.tile([P, 1], F32, tag="bflag")
                    nc.vector.tensor_single_scalar(bflag[:], cur[t][:], -128.0,
                                                   Alu.is_le)
                    pbf = paux.tile([1, P], F32, tag="tp")
                    nc.tensor.transpose(pbf[:], bflag[:], ident[:])
                    bsb = smalls.tile([1, P], BF16, tag="bsb")
                    nc.scalar.activation(bsb[:], pbf[:], Act.Copy)
                    nc.sync.dma_start(augw[t][1:2, :], bsb[:])
                    pd = [pdist.tile([P, 512], F32, tag="pd", name=R+f"pd{q}_{t}_{ch}")
                          for ch in range(4)]
                    for pair in ((0, 1), (2, 3)):
                        for m in range(2):
                            for ch in pair:
                                nc.tensor.matmul(pd[ch][:], rT[m][t][:],
                                                 e2T[m][:, ch * 512:(ch + 1) * 512],
                                                 start=(m == 0), stop=False)
                        for ch in pair:
                            nc.tensor.matmul(pd[ch][:], augw[t][:],
                                             eneg[:, ch * 512:(ch + 1) * 512],
                                             start=False, stop=True)
                    s1 = work.tile([P, K], F32, tag="s1")
                    for ch in range(4):
                        nc.scalar.activation(s1[:, ch * 512:(ch + 1) * 512], pd[ch][:],
                                             Act.Identity, bias=cur[t][:], scale=1.0)
                    m8 = smalls.tile([P, 8], F32, tag="m8")
                    nc.vector.max(m8[:], s1[:])
                    idx = smalls.tile([P, 8], U32, tag="idx")
                    nc.vector.max_index(idx[:], m8[:], s1[:])
                    # next layer bias = max(s1) = -dist_min = -x_sq_next
                    nc.scalar.activation(nxt[t][:], m8[:, 0:1], Act.Copy)
                    idxg = smalls.tile([P, 1], U32, tag="idxg")
                    nc.vector.tensor_single_scalar(idxg[:], idx[:, 0:1], float(q * K), Alu.add)
                    qrow = smalls.tile([P, D_CB], F32, tag="qrow")
                    if NO_GATHER:
                        nc.sync.dma_start(qrow[:], emb[q * K:q * K + P, :])
                    else:
                        nc.gpsimd.indirect_dma_start(
                            out=qrow[:], out_offset=None, in_=emb[:, :],
                            in_offset=bass.IndirectOffsetOnAxis(ap=idxg[:, 0:1], axis=0))
                    ptq = pqt.tile([P, D_CB], F32, tag="ptq")
                    for m in range(2):
                        nc.tensor.transpose(ptq[:, m * P:(m + 1) * P],
                                            qrow[:, m * P:(m + 1) * P], ident[:])
                    for m in range(2):
                        nc.vector.tensor_tensor(rT[m][t][:], rT[m][t][:],
                                                ptq[:, m * P:(m + 1) * P], op=Alu.subtract)
                        nc.vector.tensor_tensor(outT[m][t][:], outT[m][t][:],
                                                ptq[:, m * P:(m + 1) * P], op=Alu.add)

            # ---------------- output projection ----------------
            for t in range(NT):
                py = pdist.tile([P, D_IN], F32, tag="pd")
                for m in range(2):
                    nc.tensor.matmul(py[:], outT[m][t][:], w_out_T[:, m, :],
                                     start=(m == 0), stop=(m == 1))
                ysb = work.tile([P, D_IN], F32, tag="ysb")
                nc.scalar.activation(ysb[:], py[:], Act.Copy)
                nc.sync.dma_start(y[t * P:(t + 1) * P, :], ysb[:])

    nc.compile()
    return nc


_NC_CACHE = None


def _get_nc(reps=1):
    global _NC_CACHE
    if _NC_CACHE is None:
        _NC_CACHE = _build(reps)
    return _NC_CACHE


def kernel(x_td, w_in, w_out, embeddings, _trace=False):
    x_td = np.ascontiguousarray(np.asarray(x_td, dtype=np.float32))
    w_in = np.ascontiguousarray(np.asarray(w_in, dtype=np.float32))
    w_out = np.ascontiguousarray(np.asarray(w_out, dtype=np.float32))
    emb2d = np.ascontiguousarray(
        np.asarray(embeddings, dtype=np.float32).reshape(Q * K, D_CB))

    nc = _get_nc()
    in_maps = [
        {"x": x_td[i * T_LOC:(i + 1) * T_LOC], "w_in": w_in, "w_out": w_out,
         "emb": emb2d}
        for i in range(N_CORES)
    ]
    res = run_bass_kernel_spmd(nc, in_maps, core_ids=list(range(N_CORES)),
                               trace=_trace)
    out = np.concatenate([r["y"] for r in res.results], axis=0)
    if _trace:
        kernel.last_exec_time_ns = res.exec_time_ns
        kernel.last_results = res
    return out


if __name__ == "__main__":
    rng = np.random.default_rng(0)
    xs = rng.standard_normal((T, D_IN)).astype(np.float32)
    wi = rng.uniform(-1, 1, (D_CB, D_IN)).astype(np.float32) / np.sqrt(D_IN)
    wo = rng.uniform(-1, 1, (D_IN, D_CB)).astype(np.float32) / np.sqrt(D_CB)
    em = (rng.uniform(-1, 1, (Q, K, D_CB)).astype(np.float32) / K)
    out = kernel(xs, wi, wo, em)
    print("kernel ran, out", out.shape, out.dtype, float(np.abs(out).max()))



# revision 2
# speedup vs baseline: 1.4802x; 1.4802x over previous
"""Residual VQ (Mimi) kernel for 8x TRN2 NeuronCores.

Data-parallel over time: each core processes T/8 = 4096 timesteps.

Numerics contract: the graded reference runs jax-on-neuron, whose
distance expression rounds as fl(fl(x_sq - 2c) + e_sq) with fp32 PE
matmuls. We reproduce that structure:
  - cross 2c via 3-term bf16 decomposition (r1 e1 + r1 e2 + r2 e1),
    which matches the fp32 PE cross to ~1e-8 (measured end-to-end
    rel err 0.0048 vs device reference).
  - the fl(fl(x_sq - 2c) + e_sq) rounding + argmin happen inside ONE
    custom DVE instruction (scan-MIN + first-index accumulation).
    Codebooks are stored k-REVERSED so accum=MAX yields the FIRST
    original index on ties, matching jnp.argmin.
  - x_sq recomputed per layer (ACT Square + accum) — insensitive to
    summation order (validated numerically).

Per-core engine budget (256 tile-layer iterations):
  PE  ~ 24 bf16 matmuls (12.3k cyc) + 2 transposes  -> ~1.4 ms
  DVE ~ 1 fused argmin pass (2k cyc) + 2 small ops  -> ~0.7 ms
  ACT ~ r1 split, x_sq, evacuations                 -> ~0.4 ms
  Pool~ gather + natural-residual update            -> ~0.6 ms
"""
import numpy as np
import ml_dtypes

import concourse.bacc as bacc
import concourse.bass as bass
import concourse.mybir as mybir
import concourse.tile as tile
from concourse.bass_utils import run_bass_kernel_spmd
from concourse.masks import make_identity

from concourse import dve_ops
from concourse.dve_spec import (
    Spec, Src0, Src1, Idx, MaxNeg, scan, select, eq, lower, AluOp,
    _has_src1 as has_src1,
)
from concourse.dve_uop import DveOpSpec

F32 = mybir.dt.float32
BF16 = mybir.dt.bfloat16
U32 = mybir.dt.uint32

T, D_IN, D_CB, K, Q = 32768, 512, 256, 2048, 8
N_CORES = 8
T_LOC = T // N_CORES          # 4096
NT = T_LOC // 128             # 32 t-tiles
P = 128

Act = mybir.ActivationFunctionType
Alu = mybir.AluOpType


def _register_op(name, spec):
    existing = {op.name: op for op in dve_ops.OPS}
    if name in existing:
        return existing[name]
    row = dve_ops._CUSTOM_DVE_ROW_BASE + len(dve_ops.OPS)
    assert row < 0x20
    shas = {}
    for ver in ("v3", "v4"):
        uops = lower(spec, ver=ver)
        shas[ver] = DveOpSpec(name=name, opcode=row, uops=uops,
                              rd1_en=has_src1(spec)).sha(ver)
    op = dve_ops.DveOp(name, spec, subdim=False, uops_sha=shas)
    dve_ops.OPS.append(op)
    dve_ops.CUSTOM_DVE_SPECS[name] = spec
    dve_ops._SUB_OPCODE_FOR_NAME[name] = row
    return op


def _make_vq_argmin_op():
    """t2 = fl(fl(C0 - Src0) + Src1); running-min scan; accum = MAX of
    indices where t2 equals the running min = last improvement = (with
    k-reversed data) the FIRST original index achieving the min."""
    from concourse.dve_spec import C0, Zero
    tt1 = C0 - Src0
    tt2 = tt1 + Src1
    m = scan(AluOp.MIN, tt2, init=Zero - MaxNeg)
    body = select(eq(tt2, m), Idx, MaxNeg)
    return _register_op("VQ_ARGMIN_GRID", Spec(body=body, accum=AluOp.MAX))


def _build():
    op_argmin = _make_vq_argmin_op()

    nc = bacc.Bacc(None, target_bir_lowering=False, num_swdge_queues=4)

    x = nc.declare_dram_parameter("x", [T_LOC, D_IN], F32, isOutput=False)
    w_in = nc.declare_dram_parameter("w_in", [D_CB, D_IN], F32, isOutput=False)
    w_out = nc.declare_dram_parameter("w_out", [D_IN, D_CB], F32, isOutput=False)
    emb = nc.declare_dram_parameter("emb", [Q * K, D_CB], F32, isOutput=False)
    # host-preprocessed, k-REVERSED, transposed bf16 term tables + esq rows
    e1t = nc.declare_dram_parameter("e1t", [Q, D_CB, K], BF16, isOutput=False)
    e2t = nc.declare_dram_parameter("e2t", [Q, D_CB, K], BF16, isOutput=False)
    esqb = nc.declare_dram_parameter("esqb", [Q, P, K], F32, isOutput=False)
    y = nc.declare_dram_parameter("y", [T_LOC, D_IN], F32, isOutput=True)

    with tile.TileContext(nc) as tc:
        with (
            tc.tile_pool(name="const", bufs=1) as constp,
            tc.tile_pool(name="state", bufs=1) as state,
            tc.tile_pool(name="elay", bufs=2) as elay,
            tc.tile_pool(name="rsplit", bufs=3) as rsplit,
            tc.tile_pool(name="smalls", bufs=6) as smalls,
            tc.tile_pool(name="dumpp", bufs=2) as dumpp,
            tc.tile_pool(name="qrowp", bufs=4) as qrowp,
            tc.tile_pool(name="ysbp", bufs=2) as ysbp,
            tc.tile_pool(name="pscore", bufs=2, space="PSUM") as pscore,
        ):
            ident = constp.tile([P, P], F32, tag="ident")
            make_identity(nc, ident[:])

            w_in_T = constp.tile([P, 4, D_CB], F32, tag="w_in_T")   # [din_p, din_c, dcb]
            w_out_T = constp.tile([P, 2, D_IN], F32, tag="w_out_T")  # [dcb_p, dcb_c, dout]

            # state: rT / r0T [128 dcb-part, (m,t) 256], r_nat [128 t, 256 dcb]
            rT = [state.tile([P, 2 * P], F32, tag=f"rT{t}", name=f"rT{t}") for t in range(NT)]
            r0T = [state.tile([P, 2 * P], F32, tag=f"r0T{t}", name=f"r0T{t}") for t in range(NT)]
            rnat = [state.tile([P, D_CB], F32, tag=f"rn{t}", name=f"rn{t}") for t in range(NT)]
            xsq = [state.tile([P, 1], F32, tag=f"xq{t}", name=f"xq{t}") for t in range(NT)]

            # ---------------- init: weight transposes ----------------
            with tc.tile_pool(name="initp", bufs=1) as initp:
                wtmp = initp.tile([P, 2, D_IN], F32, tag="wtmp")
                nc.sync.dma_start(wtmp[:], w_in[:].rearrange("(c p) d -> p c d", p=P))
                for ci in range(4):
                    for m in range(2):
                        tp = pscore.tile([P, K], F32, tag="sc")
                        nc.tensor.transpose(tp[:, 0:P], wtmp[:, m, ci * P:(ci + 1) * P], ident[:])
                        nc.scalar.activation(w_in_T[:, ci, m * P:(m + 1) * P], tp[:, 0:P], Act.Copy)
                wtmp2 = initp.tile([P, 4, D_CB], F32, tag="wtmp")
                nc.sync.dma_start(wtmp2[:], w_out[:].rearrange("(c p) d -> p c d", p=P))
                for ci in range(4):
                    for m in range(2):
                        tp = pscore.tile([P, K], F32, tag="sc")
                        nc.tensor.transpose(tp[:, 0:P], wtmp2[:, ci, m * P:(m + 1) * P], ident[:])
                        nc.scalar.activation(w_out_T[:, m, ci * P:(ci + 1) * P], tp[:, 0:P], Act.Copy)

                # ---------------- init: x -> r0 (both layouts), x_sq ----------------
                for b in range(8):  # 512-t blocks
                    xblk = initp.tile([P, 4, D_IN], F32, tag="xblk")
                    nc.sync.dma_start(
                        xblk[:], x[b * 512:(b + 1) * 512, :].rearrange("(c p) d -> p c d", p=P))
                    xT = initp.tile([P, 4, 512], F32, tag="xT")  # [din_p, din_c, t_in_blk]
                    for tb in range(4):
                        for db in range(4):
                            tp = pscore.tile([P, K], F32, tag="sc")
                            nc.tensor.transpose(tp[:, 0:P], xblk[:, tb, db * P:(db + 1) * P], ident[:])
                            nc.scalar.activation(xT[:, db, tb * P:(tb + 1) * P], tp[:, 0:P], Act.Copy)
                    # rT chunks: [dcb_m, t] for 512 t
                    for m in range(2):
                        pr = pscore.tile([P, K], F32, tag="sc")
                        for ci in range(4):
                            nc.tensor.matmul(pr[:, 0:512], w_in_T[:, ci, m * P:(m + 1) * P],
                                             xT[:, ci, :], start=(ci == 0), stop=(ci == 3))
                        for tb in range(4):
                            t = b * 4 + tb
                            nc.scalar.activation(rT[t][:, m * P:(m + 1) * P],
                                                 pr[:, tb * P:(tb + 1) * P], Act.Copy)
                            nc.vector.tensor_copy(r0T[t][:, m * P:(m + 1) * P],
                                                  pr[:, tb * P:(tb + 1) * P])
                    # natural r per t-subtile -> r_nat, x_sq
                    for tb in range(4):
                        t = b * 4 + tb
                        pn = pscore.tile([P, K], F32, tag="sc")
                        for ci in range(4):
                            nc.tensor.matmul(pn[:, 0:D_CB], xT[:, ci, tb * P:(tb + 1) * P],
                                             w_in_T[:, ci, :], start=(ci == 0), stop=(ci == 3))
                        nc.scalar.activation(rnat[t][:], pn[:, 0:D_CB], Act.Copy)
                        sqj = initp.tile([P, D_CB], BF16, tag="sqj")
                        nc.scalar.activation(sqj[:], rnat[t][:], Act.Square,
                                             accum_out=xsq[t][:])

            # ---------------- main: 8 codebook layers ----------------
            for q in range(Q):
                e1 = elay.tile([P, 2, K], BF16, tag="e1", name=f"e1_{q}")
                nc.sync.dma_start(e1[:], e1t[q].rearrange("(m p) k -> p m k", p=P))
                e2 = elay.tile([P, 2, K], BF16, tag="e2", name=f"e2_{q}")
                nc.sync.dma_start(e2[:], e2t[q].rearrange("(m p) k -> p m k", p=P))
                esq = elay.tile([P, K], F32, tag="esq", name=f"esq_{q}")
                nc.sync.dma_start(esq[:], esqb[q])

                for t in range(NT):
                    # r split: r1 = bf16(rT), r2 = bf16(rT - r1)
                    r1 = rsplit.tile([P, 2 * P], BF16, tag="r1", name=f"r1_{q}_{t}")
                    nc.scalar.activation(r1[:], rT[t][:], Act.Copy)
                    r2 = rsplit.tile([P, 2 * P], BF16, tag="r2", name=f"r2_{q}_{t}")
                    nc.vector.tensor_tensor(r2[:], rT[t][:], r1[:], op=Alu.subtract)

                    S = pscore.tile([P, K], F32, tag="sc")
                    terms = [(r1, e1), (r1, e2), (r2, e1)]
                    ntm = len(terms) * 2
                    ti = 0
                    for (rt_, et_) in terms:
                        for m in range(2):
                            for ch in range(4):
                                nc.tensor.matmul(
                                    S[:, ch * 512:(ch + 1) * 512],
                                    rt_[:, m * P:(m + 1) * P],
                                    et_[:, m, ch * 512:(ch + 1) * 512],
                                    start=(ti == 0), stop=(ti == ntm - 1))
                            ti += 1

                    # fused fl(fl(xsq - 2c) + esq) + argmin (first-index via reversal)
                    dump = dumpp.tile([P, K], F32, tag="dump")
                    jstar = smalls.tile([P, 1], F32, tag="jstar")
                    nc.vector._custom_dve(
                        op_argmin, out=dump[:], in0=S[:],
                        in1=esq[:].unsqueeze(1), s0=xsq[t][:],
                        accum_out=jstar[:])

                    # original index = (2047 + q*2048) - jstar
                    jneg = smalls.tile([P, 1], F32, tag="jneg")
                    nc.vector.tensor_scalar(
                        jneg[:], jstar[:], -1.0, float(K - 1 + q * K),
                        op0=Alu.mult, op1=Alu.add)
                    idxg = smalls.tile([P, 1], U32, tag="idxg")
                    nc.vector.tensor_copy(idxg[:], jneg[:])

                    qrow = qrowp.tile([P, D_CB], F32, tag="qrow")
                    nc.gpsimd.indirect_dma_start(
                        out=qrow[:], out_offset=None, in_=emb[:, :],
                        in_offset=bass.IndirectOffsetOnAxis(ap=idxg[:, 0:1], axis=0))

                    # transposed quant into the (now-consumed) score tile
                    for m in range(2):
                        nc.tensor.transpose(S[:, m * P:(m + 1) * P],
                                            qrow[:, m * P:(m + 1) * P], ident[:])
                    nc.vector.tensor_tensor(rT[t][:], rT[t][:], S[:, 0:2 * P],
                                            op=Alu.subtract)
                    # natural-layout residual + x_sq for next layer
                    nc.gpsimd.tensor_tensor(rnat[t][:], rnat[t][:], qrow[:],
                                            op=Alu.subtract)
                    if q < Q - 1:
                        sqj = rsplit.tile([P, D_CB], BF16, tag="sqj2",
                                          name=f"sqj_{q}_{t}")
                        nc.scalar.activation(sqj[:], rnat[t][:], Act.Square,
                                             accum_out=xsq[t][:])

            # ---------------- output projection: out = r0 - r8 ----------------
            for t in range(NT):
                nc.vector.tensor_tensor(r0T[t][:], r0T[t][:], rT[t][:],
                                        op=Alu.subtract)
                py = pscore.tile([P, K], F32, tag="sc")
                for m in range(2):
                    nc.tensor.matmul(py[:, 0:D_IN], r0T[t][:, m * P:(m + 1) * P],
                                     w_out_T[:, m, :], start=(m == 0), stop=(m == 1))
                ysb = ysbp.tile([P, D_IN], F32, tag="ysb")
                nc.scalar.activation(ysb[:], py[:, 0:D_IN], Act.Copy)
                nc.sync.dma_start(y[t * P:(t + 1) * P, :], ysb[:])

    nc.compile()
    return nc


_NC_CACHE = None


def _get_nc():
    global _NC_CACHE
    if _NC_CACHE is None:
        _NC_CACHE = _build()
    return _NC_CACHE


def _round_bf16(x):
    return x.astype(ml_dtypes.bfloat16)


def kernel(x_td, w_in, w_out, embeddings, _trace=False):
    x_td = np.ascontiguousarray(np.asarray(x_td, dtype=np.float32))
    w_in = np.ascontiguousarray(np.asarray(w_in, dtype=np.float32))
    w_out = np.ascontiguousarray(np.asarray(w_out, dtype=np.float32))
    emb3 = np.asarray(embeddings, dtype=np.float32)
    emb2d = np.ascontiguousarray(emb3.reshape(Q * K, D_CB))

    # host preprocessing: k-reversed, doubled, bf16-split, transposed tables
    erev = emb3[:, ::-1, :]                           # [Q, K, D] reversed k
    e2x = (2.0 * erev).astype(np.float32)
    e1 = _round_bf16(e2x)
    e2 = _round_bf16(e2x - e1.astype(np.float32))
    e1t = np.ascontiguousarray(np.asarray(e1).transpose(0, 2, 1))   # [Q, D, K] bf16
    e2t = np.ascontiguousarray(np.asarray(e2).transpose(0, 2, 1))
    esq = (erev.astype(np.float32) ** 2).sum(axis=2, dtype=np.float32)  # [Q, K]
    esqb = np.ascontiguousarray(
        np.broadcast_to(esq[:, None, :], (Q, P, K)).astype(np.float32))

    nc = _get_nc()
    in_maps = [
        {"x": x_td[i * T_LOC:(i + 1) * T_LOC], "w_in": w_in, "w_out": w_out,
         "emb": emb2d, "e1t": e1t, "e2t": e2t, "esqb": esqb}
        for i in range(N_CORES)
    ]
    res = run_bass_kernel_spmd(nc, in_maps, core_ids=list(range(N_CORES)),
                               trace=_trace)
    out = np.concatenate([r["y"] for r in res.results], axis=0)
    if _trace:
        kernel.last_exec_time_ns = res.exec_time_ns
        kernel.last_results = res
    return out


if __name__ == "__main__":
    rng = np.random.default_rng(0)
    xs = rng.standard_normal((T, D_IN)).astype(np.float32)
    wi = rng.uniform(-1, 1, (D_CB, D_IN)).astype(np.float32) / np.sqrt(D_IN)
    wo = rng.uniform(-1, 1, (D_IN, D_CB)).astype(np.float32) / np.sqrt(D_CB)
    em = (rng.uniform(-1, 1, (Q, K, D_CB)).astype(np.float32) / K)
    out = kernel(xs, wi, wo, em)
    print("kernel ran, out", out.shape, out.dtype, float(np.abs(out).max()))


# revision 3
# speedup vs baseline: 1.5840x; 1.0701x over previous
"""Residual VQ (Mimi) kernel for 8x TRN2 NeuronCores.

Data-parallel over time: each core processes T/8 = 4096 timesteps.

Numerics contract: the graded reference runs jax-on-neuron, whose
distance expression rounds as fl(fl(x_sq - 2c) + e_sq) with fp32 PE
matmuls. We reproduce that structure:
  - cross 2c via 3-term bf16 decomposition (r1 e1 + r1 e2 + r2 e1),
    which matches the fp32 PE cross to ~1e-8 (measured end-to-end
    rel err 0.0048 vs device reference).
  - the fl(fl(x_sq - 2c) + e_sq) rounding + argmin happen inside ONE
    custom DVE instruction (scan-MIN + first-index accumulation).
    Codebooks are stored k-REVERSED so accum=MAX yields the FIRST
    original index on ties, matching jnp.argmin.
  - x_sq recomputed per layer (ACT Square + accum) — insensitive to
    summation order (validated numerically).

Per-core engine budget (256 tile-layer iterations):
  PE  ~ 24 bf16 matmuls (12.3k cyc) + 2 transposes  -> ~1.4 ms
  DVE ~ 1 fused argmin pass (2k cyc) + 2 small ops  -> ~0.7 ms
  ACT ~ r1 split, x_sq, evacuations                 -> ~0.4 ms
  Pool~ gather + natural-residual update            -> ~0.6 ms
"""
import numpy as np
import ml_dtypes

import concourse.bacc as bacc
import concourse.bass as bass
import concourse.mybir as mybir
import concourse.tile as tile
from concourse.bass_utils import run_bass_kernel_spmd
from concourse.masks import make_identity

from concourse import dve_ops
from concourse.dve_spec import (
    Spec, Src0, Src1, Idx, MaxNeg, scan, select, eq, lower, AluOp,
    _has_src1 as has_src1,
)
from concourse.dve_uop import DveOpSpec

F32 = mybir.dt.float32
BF16 = mybir.dt.bfloat16
U32 = mybir.dt.uint32

T, D_IN, D_CB, K, Q = 32768, 512, 256, 2048, 8
N_CORES = 8
T_LOC = T // N_CORES          # 4096
NT = T_LOC // 128             # 32 t-tiles
P = 128

Act = mybir.ActivationFunctionType
Alu = mybir.AluOpType


def _register_op(name, spec):
    existing = {op.name: op for op in dve_ops.OPS}
    if name in existing:
        return existing[name]
    row = dve_ops._CUSTOM_DVE_ROW_BASE + len(dve_ops.OPS)
    assert row < 0x20
    shas = {}
    for ver in ("v3", "v4"):
        uops = lower(spec, ver=ver)
        shas[ver] = DveOpSpec(name=name, opcode=row, uops=uops,
                              rd1_en=has_src1(spec)).sha(ver)
    op = dve_ops.DveOp(name, spec, subdim=False, uops_sha=shas)
    dve_ops.OPS.append(op)
    dve_ops.CUSTOM_DVE_SPECS[name] = spec
    dve_ops._SUB_OPCODE_FOR_NAME[name] = row
    return op


def _make_vq_argmin_op():
    """t2 = fl(fl(C0 - Src0) + Src1); running-min scan; accum = MAX of
    indices where t2 equals the running min = last improvement = (with
    k-reversed data) the FIRST original index achieving the min."""
    from concourse.dve_spec import C0, Zero
    tt1 = C0 - Src0
    tt2 = tt1 + Src1
    m = scan(AluOp.MIN, tt2, init=Zero - MaxNeg)
    body = select(eq(tt2, m), Idx, MaxNeg)
    return _register_op("VQ_ARGMIN_GRID", Spec(body=body, accum=AluOp.MAX))


def _build():
    op_argmin = _make_vq_argmin_op()

    nc = bacc.Bacc(None, target_bir_lowering=False, num_swdge_queues=4)

    x = nc.declare_dram_parameter("x", [T_LOC, D_IN], F32, isOutput=False)
    w_in = nc.declare_dram_parameter("w_in", [D_CB, D_IN], F32, isOutput=False)
    w_out = nc.declare_dram_parameter("w_out", [D_IN, D_CB], F32, isOutput=False)
    emb = nc.declare_dram_parameter("emb", [Q * K, D_CB], F32, isOutput=False)
    # host-preprocessed, k-REVERSED, transposed bf16 term tables + esq rows
    e1t = nc.declare_dram_parameter("e1t", [Q, D_CB, K], BF16, isOutput=False)
    e2t = nc.declare_dram_parameter("e2t", [Q, D_CB, K], BF16, isOutput=False)
    esqb = nc.declare_dram_parameter("esqb", [Q, P, K], F32, isOutput=False)
    y = nc.declare_dram_parameter("y", [T_LOC, D_IN], F32, isOutput=True)

    with tile.TileContext(nc) as tc:
        with (
            tc.tile_pool(name="const", bufs=1) as constp,
            tc.tile_pool(name="state", bufs=1) as state,
            tc.tile_pool(name="elay", bufs=2) as elay,
            tc.tile_pool(name="rsplit", bufs=2) as rsplit,
            tc.tile_pool(name="smalls", bufs=6) as smalls,
            tc.tile_pool(name="dumpp", bufs=1) as dumpp,
            tc.tile_pool(name="qrowp", bufs=33) as qrowp,
            tc.tile_pool(name="ysbp", bufs=1) as ysbp,
            tc.tile_pool(name="pscore", bufs=2, space="PSUM") as pscore,
        ):
            ident = constp.tile([P, P], F32, tag="ident")
            make_identity(nc, ident[:])

            w_in_T = constp.tile([P, 4, D_CB], F32, tag="w_in_T")   # [din_p, din_c, dcb]
            w_out_T = constp.tile([P, 2, D_IN], F32, tag="w_out_T")  # [dcb_p, dcb_c, dout]

            # state: rT / r0T [128 dcb-part, (m,t) 256], r_nat [128 t, 256 dcb]
            rT = [state.tile([P, 2 * P], F32, tag=f"rT{t}", name=f"rT{t}") for t in range(NT)]
            r0T = [state.tile([P, 2 * P], F32, tag=f"r0T{t}", name=f"r0T{t}") for t in range(NT)]
            rnat = [state.tile([P, D_CB], F32, tag=f"rn{t}", name=f"rn{t}") for t in range(NT)]
            xsq = [state.tile([P, 1], F32, tag=f"xq{t}", name=f"xq{t}") for t in range(NT)]

            # ---------------- init: weight transposes ----------------
            with tc.tile_pool(name="initp", bufs=1) as initp:
                wtmp = initp.tile([P, 2, D_IN], F32, tag="wtmp")
                nc.sync.dma_start(wtmp[:], w_in[:].rearrange("(c p) d -> p c d", p=P))
                for ci in range(4):
                    for m in range(2):
                        tp = pscore.tile([P, K], F32, tag="sc")
                        nc.tensor.transpose(tp[:, 0:P], wtmp[:, m, ci * P:(ci + 1) * P], ident[:])
                        nc.scalar.activation(w_in_T[:, ci, m * P:(m + 1) * P], tp[:, 0:P], Act.Copy)
                wtmp2 = initp.tile([P, 4, D_CB], F32, tag="wtmp")
                nc.sync.dma_start(wtmp2[:], w_out[:].rearrange("(c p) d -> p c d", p=P))
                for ci in range(4):
                    for m in range(2):
                        tp = pscore.tile([P, K], F32, tag="sc")
                        nc.tensor.transpose(tp[:, 0:P], wtmp2[:, ci, m * P:(m + 1) * P], ident[:])
                        nc.scalar.activation(w_out_T[:, m, ci * P:(ci + 1) * P], tp[:, 0:P], Act.Copy)

                # ---------------- init: x -> r0 (both layouts), x_sq ----------------
                for b in range(16):  # 256-t blocks
                    xblk = initp.tile([P, 2, D_IN], F32, tag="wtmp")
                    nc.sync.dma_start(
                        xblk[:], x[b * 256:(b + 1) * 256, :].rearrange("(c p) d -> p c d", p=P))
                    xT = initp.tile([P, 4, 256], F32, tag="xT")  # [din_p, din_c, t_in_blk]
                    for tb in range(2):
                        for db in range(4):
                            tp = pscore.tile([P, K], F32, tag="sc")
                            nc.tensor.transpose(tp[:, 0:P], xblk[:, tb, db * P:(db + 1) * P], ident[:])
                            nc.scalar.activation(xT[:, db, tb * P:(tb + 1) * P], tp[:, 0:P], Act.Copy)
                    # rT chunks: [dcb_m, t] for 256 t
                    for m in range(2):
                        pr = pscore.tile([P, K], F32, tag="sc")
                        for ci in range(4):
                            nc.tensor.matmul(pr[:, 0:256], w_in_T[:, ci, m * P:(m + 1) * P],
                                             xT[:, ci, :], start=(ci == 0), stop=(ci == 3))
                        for tb in range(2):
                            t = b * 2 + tb
                            nc.scalar.activation(rT[t][:, m * P:(m + 1) * P],
                                                 pr[:, tb * P:(tb + 1) * P], Act.Copy)
                            nc.vector.tensor_copy(r0T[t][:, m * P:(m + 1) * P],
                                                  pr[:, tb * P:(tb + 1) * P])
                    # natural r per t-subtile -> r_nat, x_sq
                    for tb in range(2):
                        t = b * 2 + tb
                        pn = pscore.tile([P, K], F32, tag="sc")
                        for ci in range(4):
                            nc.tensor.matmul(pn[:, 0:D_CB], xT[:, ci, tb * P:(tb + 1) * P],
                                             w_in_T[:, ci, :], start=(ci == 0), stop=(ci == 3))
                        nc.scalar.activation(rnat[t][:], pn[:, 0:D_CB], Act.Copy)
                        sqj = initp.tile([P, D_CB], BF16, tag="sqj")
                        nc.scalar.activation(sqj[:], rnat[t][:], Act.Square,
                                             accum_out=xsq[t][:])

            # ---------------- main: 8 codebook layers ----------------
            # Per-window work is PURE matmuls on the PE: the argmin/gather
            # chain runs on DVE/Pool, and the quant transpose + rT update
            # for ALL 32 tiles is batched at the END of the layer (the rT
            # deadline is the next layer's same tile, ~165us away). This
            # keeps the PE FIFO free of gather-dependent work so the HAM
            # clock-gate ramps to 2.4 GHz and stays there per layer.
            def stage_layer(qq):
                e1_ = elay.tile([P, 2, K], BF16, tag="e1", name=f"e1_{qq}")
                nc.sync.dma_start(e1_[:], e1t[qq].rearrange("(m p) k -> p m k", p=P))
                e2_ = elay.tile([P, 2, K], BF16, tag="e2", name=f"e2_{qq}")
                nc.sync.dma_start(e2_[:], e2t[qq].rearrange("(m p) k -> p m k", p=P))
                esq_ = elay.tile([P, K], F32, tag="esq", name=f"esq_{qq}")
                nc.sync.dma_start(esq_[:], esqb[qq])
                return (e1_, e2_, esq_)

            staged = stage_layer(0)
            for q in range(Q):
                e1, e2, esq = staged
                qrows = []
                for t in range(NT):
                    if q < Q - 1 and t == 8:
                        staged = stage_layer(q + 1)
                    # r split: r1 = bf16(rT), r2 = bf16(rT - r1)
                    r1 = rsplit.tile([P, 2 * P], BF16, tag="r1", name=f"r1_{q}_{t}")
                    nc.scalar.activation(r1[:], rT[t][:], Act.Copy)
                    r2 = rsplit.tile([P, 2 * P], BF16, tag="r2", name=f"r2_{q}_{t}")
                    nc.vector.tensor_tensor(r2[:], rT[t][:], r1[:], op=Alu.subtract)

                    S = pscore.tile([P, K], F32, tag="sc")
                    terms = [(r1, e1), (r1, e2), (r2, e1)]
                    ntm = len(terms) * 2
                    ti = 0
                    for (rt_, et_) in terms:
                        for m in range(2):
                            for ch in range(4):
                                nc.tensor.matmul(
                                    S[:, ch * 512:(ch + 1) * 512],
                                    rt_[:, m * P:(m + 1) * P],
                                    et_[:, m, ch * 512:(ch + 1) * 512],
                                    start=(ti == 0), stop=(ti == ntm - 1))
                            ti += 1

                    # fused fl(fl(xsq - 2c) + esq) + argmin (first-index via reversal)
                    dump = dumpp.tile([P, K], BF16, tag="dump")
                    jstar = smalls.tile([P, 1], F32, tag="jstar")
                    nc.vector._custom_dve(
                        op_argmin, out=dump[:], in0=S[:],
                        in1=esq[:].unsqueeze(1), s0=xsq[t][:],
                        accum_out=jstar[:])

                    # original index = (2047 + q*2048) - jstar
                    jneg = smalls.tile([P, 1], F32, tag="jneg")
                    nc.vector.tensor_scalar(
                        jneg[:], jstar[:], -1.0, float(K - 1 + q * K),
                        op0=Alu.mult, op1=Alu.add)
                    idxg = smalls.tile([P, 1], U32, tag="idxg")
                    nc.vector.tensor_copy(idxg[:], jneg[:])

                    qrow = qrowp.tile([P, D_CB], F32, tag="qrow",
                                      name=f"qrow_{q}_{t}")
                    nc.gpsimd.indirect_dma_start(
                        out=qrow[:], out_offset=None, in_=emb[:, :],
                        in_offset=bass.IndirectOffsetOnAxis(ap=idxg[:, 0:1], axis=0))
                    qrows.append(qrow)

                    # natural-layout residual + x_sq for next layer (Pool/ACT)
                    if q < Q - 1:
                        nc.gpsimd.tensor_tensor(rnat[t][:], rnat[t][:], qrow[:],
                                                op=Alu.subtract)
                        sqj = rsplit.tile([P, D_CB], BF16, tag="sqj2",
                                          name=f"sqj_{q}_{t}")
                        nc.scalar.activation(sqj[:], rnat[t][:], Act.Square,
                                             accum_out=xsq[t][:])

                # layer-end batch: transpose each tile's quant and update rT
                for t in range(NT):
                    ptq = pscore.tile([P, K], F32, tag="sc")
                    for m in range(2):
                        nc.tensor.transpose(ptq[:, m * P:(m + 1) * P],
                                            qrows[t][:, m * P:(m + 1) * P],
                                            ident[:])
                    nc.vector.tensor_tensor(rT[t][:], rT[t][:], ptq[:, 0:2 * P],
                                            op=Alu.subtract)

            # ---------------- output projection: out = r0 - r8 ----------------
            for t in range(NT):
                nc.vector.tensor_tensor(r0T[t][:], r0T[t][:], rT[t][:],
                                        op=Alu.subtract)
                py = pscore.tile([P, K], F32, tag="sc")
                for m in range(2):
                    nc.tensor.matmul(py[:, 0:D_IN], r0T[t][:, m * P:(m + 1) * P],
                                     w_out_T[:, m, :], start=(m == 0), stop=(m == 1))
                ysb = ysbp.tile([P, D_IN], F32, tag="ysb")
                nc.scalar.activation(ysb[:], py[:, 0:D_IN], Act.Copy)
                nc.sync.dma_start(y[t * P:(t + 1) * P, :], ysb[:])

    nc.compile()
    return nc


_NC_CACHE = None


def _get_nc():
    global _NC_CACHE
    if _NC_CACHE is None:
        _NC_CACHE = _build()
    return _NC_CACHE


def _round_bf16(x):
    return x.astype(ml_dtypes.bfloat16)


def kernel(x_td, w_in, w_out, embeddings, _trace=False):
    x_td = np.ascontiguousarray(np.asarray(x_td, dtype=np.float32))
    w_in = np.ascontiguousarray(np.asarray(w_in, dtype=np.float32))
    w_out = np.ascontiguousarray(np.asarray(w_out, dtype=np.float32))
    emb3 = np.asarray(embeddings, dtype=np.float32)
    emb2d = np.ascontiguousarray(emb3.reshape(Q * K, D_CB))

    # host preprocessing: k-reversed, doubled, bf16-split, transposed tables
    erev = emb3[:, ::-1, :]                           # [Q, K, D] reversed k
    e2x = (2.0 * erev).astype(np.float32)
    e1 = _round_bf16(e2x)
    e2 = _round_bf16(e2x - e1.astype(np.float32))
    e1t = np.ascontiguousarray(np.asarray(e1).transpose(0, 2, 1))   # [Q, D, K] bf16
    e2t = np.ascontiguousarray(np.asarray(e2).transpose(0, 2, 1))
    esq = (erev.astype(np.float32) ** 2).sum(axis=2, dtype=np.float32)  # [Q, K]
    esqb = np.ascontiguousarray(
        np.broadcast_to(esq[:, None, :], (Q, P, K)).astype(np.float32))

    nc = _get_nc()
    in_maps = [
        {"x": x_td[i * T_LOC:(i + 1) * T_LOC], "w_in": w_in, "w_out": w_out,
         "emb": emb2d, "e1t": e1t, "e2t": e2t, "esqb": esqb}
        for i in range(N_CORES)
    ]
    res = run_bass_kernel_spmd(nc, in_maps, core_ids=list(range(N_CORES)),
                               trace=_trace)
    out = np.concatenate([r["y"] for r in res.results], axis=0)
    if _trace:
        kernel.last_exec_time_ns = res.exec_time_ns
        kernel.last_results = res
    return out


if __name__ == "__main__":
    rng = np.random.default_rng(0)
    xs = rng.standard_normal((T, D_IN)).astype(np.float32)
    wi = rng.uniform(-1, 1, (D_CB, D_IN)).astype(np.float32) / np.sqrt(D_IN)
    wo = rng.uniform(-1, 1, (D_IN, D_CB)).astype(np.float32) / np.sqrt(D_CB)
    em = (rng.uniform(-1, 1, (Q, K, D_CB)).astype(np.float32) / K)
    out = kernel(xs, wi, wo, em)
    print("kernel ran, out", out.shape, out.dtype, float(np.abs(out).max()))
